# revision 16
# baseline (speedup 1.0000x reference)
"""Trainium2 Bass kernel for nn_AttentionSubModule (B=262144, Q=25, D=9).

Strategy (pure data parallel over 8 NeuronCores, 32768 elements/core):
  - batch-on-partitions layout, chunks of 128 elements
  - PE: transpose x -> fullT, static block-diag projection matmuls (K, V with
    bias folded via a ones-row), transpose K/V back to batch layout
  - DVE: batched scores = K K^T (broadcast multiply + reduce), attn @ V,
    softmax normalization, residual + LayerNorm
  - ACT: exp (row-max subtracted), sqrt
Dtypes: fp32 I/O and accumulations; fp16 internal operands.
"""

import numpy as np

import bass_rust as br
import concourse.bass as bass
import concourse.mybir as mybir
import concourse.tile as tile
from concourse.bass_utils import run_bass_kernel_spmd
from concourse.vector_clock import ScopedClock

B, Q, D = 262144, 25, 9
SEGS = [(0, 3), (3, 13), (13, 23), (23, 25)]
EPS = 1e-5
N_CORES = 8
BC = B // N_CORES          # elements per core
CH = 128                   # elements per compute chunk
SUP = 8                    # chunks per DMA super-chunk
DP = 10                    # padded d size (even, for 16-bit 2x mode)
QP = 26                    # padded q' size
KF = Q * DP                # 250   K fullT rows / K_b free size
VF = D * QP                # 234   V fullT rows / V_b free size

F32 = mybir.dt.float32
F16 = mybir.dt.float16
AX = mybir.AxisListType
OP = mybir.AluOpType
ACTF = mybir.ActivationFunctionType


def _split_multi_waits(nc, max_waits=1):
    """walrus here rejects instructions with more than one sync-wait command.
    Hoist extra waits onto same-engine NOPs inserted just before the
    offending instruction (same-engine program order makes this equivalent)."""
    for bb in nc.main_func.blocks:
        insts = bb.instructions
        out = []
        changed = False
        for inst in insts:
            si = getattr(inst, "sync_info", None)
            if si is not None and len(si.on_wait) > max_waits:
                waits = list(si.on_wait)
                keep = waits[: max_waits]
                extra = waits[max_waits:]
                for w in extra:
                    nop = mybir.InstNoOp(
                        name=f"wsplit_{nc.next_id()}", ins=[], outs=[]
                    )
                    nop.engine = inst.engine
                    nop.sync_info = br.SyncInfo(on_wait=[w], on_update=[])
                    out.append(nop)
                inst.sync_info = br.SyncInfo(
                    on_wait=keep, on_update=list(si.on_update)
                )
                changed = True
            out.append(inst)
        if changed:
            bb.instructions = out


def _patch_tile_drain():
    """walrus here rejects >1 sync-wait on the Tile tail Drain; spread the
    waits over single-wait NOPs instead."""

    def _drain_and_barrier(self, tick_clock, wait_clock):
        nc = self.nc
        probe = nc.sync.nop(nofuse=True)
        wait_clock.add_sem_waits(
            probe.ins, ScopedClock({None: tick_clock.global_clock})
        )
        si = probe.ins.sync_info
        if si is not None and len(si.on_wait) > 1:
            waits = list(si.on_wait)
            probe.ins.sync_info = br.SyncInfo(
                on_wait=[waits[0]], on_update=list(si.on_update)
            )
            for w in waits[1:]:
                n = nc.sync.nop(nofuse=True)
                n.ins.sync_info = br.SyncInfo(on_wait=[w], on_update=[])
        nc.sync.drain()

        nc.all_engine_barrier()
        assert self.sems is not None
        popped = nc._tile_sem_poison_stack.pop()
        assert popped is self._sem_poison
        nc.clear_and_free_semaphores(list(self.sems.allocated().values()))
        nc.all_engine_barrier()

    tile.TileContext._drain_and_barrier = _drain_and_barrier


_patch_tile_drain()


def _seg_of(q):
    for si, (s, e) in enumerate(SEGS):
        if s <= q < e:
            return si
    raise ValueError(q)


def make_weights(inp):
    """Host-side packing of the static stationary matrices.

    WK [226, 250]: K-proj.  out column m=(q*10+d) [d<9], contraction row
      k=(qt*9+dp) for qt<25 plus bias row k=225.
      WK[qt*9+dp, q*10+d] = Wk_seg(q)[d, dp] * (qt==q);  WK[225, q*10+d] = bk[d]
    WV [226, 234]: V-proj in (d, q')-major output order, m=(d*26+q') [q'<25].
      WV[qt*9+dp, d*26+qp] = Wv_seg(qp)[d, dp] * (qt==qp); WV[225, ...] = bv[d]
    """
    Wk = [np.asarray(inp[n], np.float32) for n in ("W_jk", "W_ok", "W_gk", "W_bk")]
    bk = [np.asarray(inp[n], np.float32) for n in ("b_jk", "b_ok", "b_gk", "b_bk")]
    Wv = [np.asarray(inp[n], np.float32) for n in ("W_jv", "W_ov", "W_gv", "W_bv")]
    bv = [np.asarray(inp[n], np.float32) for n in ("b_jv", "b_ov", "b_gv", "b_bv")]

    WK = np.zeros((226, KF), np.float32)
    WV = np.zeros((226, VF), np.float32)
    for q in range(Q):
        s = _seg_of(q)
        for d in range(D):
            for dp in range(D):
                WK[q * D + dp, q * DP + d] = Wk[s][d, dp]
                WV[q * D + dp, d * QP + q] = Wv[s][d, dp]
            WK[225, q * DP + d] = bk[s][d]
            WV[225, d * QP + q] = bv[s][d]
    return WK, WV


def build_nc(n_super):
    """Build the single-core program processing n_super*SUP*CH elements."""
    n_el = n_super * SUP * CH
    nc = bass.Bass("TRN2", target_bir_lowering=False, debug=False)

    x_d = nc.dram_tensor("x", [n_el, Q * D], F32, kind="ExternalInput")
    y_d = nc.dram_tensor("y", [n_el, Q * D], F32, kind="ExternalOutput")
    wk_d = nc.dram_tensor("wk", [226, KF], F16, kind="ExternalInput")
    wv_d = nc.dram_tensor("wv", [226, VF], F16, kind="ExternalInput")
    id_d = nc.dram_tensor("ident", [128, 128], F16, kind="ExternalInput")
    idf_d = nc.dram_tensor("identf", [128, 128], F32, kind="ExternalInput")
    g_d = nc.dram_tensor("ln_g", [D], F32, kind="ExternalInput")
    b_d = nc.dram_tensor("ln_b", [D], F32, kind="ExternalInput")

    with tile.TileContext(nc) as tc:
        with (
            tc.tile_pool(name="singles", bufs=1) as singles,
            tc.tile_pool(name="xio", bufs=2) as xio,
            tc.tile_pool(name="yio", bufs=2) as yio,
            tc.tile_pool(name="kv", bufs=4) as kv,
            tc.tile_pool(name="big", bufs=3) as big,
            tc.tile_pool(name="small", bufs=6) as small,
            tc.tile_pool(name="ps", bufs=3, space="PSUM") as ps,
            tc.tile_pool(name="ps2", bufs=2, space="PSUM") as ps2,
        ):
            # --- static tiles -------------------------------------------------
            wka = singles.tile([128, KF], F16, tag="wka")
            wkb = singles.tile([98, KF], F16, tag="wkb")
            wva = singles.tile([128, VF], F16, tag="wva")
            wvb = singles.tile([98, VF], F16, tag="wvb")
            nc.sync.dma_start(out=wka, in_=wk_d[0:128, :])
            nc.sync.dma_start(out=wkb[0:98, :], in_=wk_d[128:226, :])
            nc.sync.dma_start(out=wva, in_=wv_d[0:128, :])
            nc.sync.dma_start(out=wvb[0:98, :], in_=wv_d[128:226, :])
            ident = singles.tile([128, 128], F16, tag="ident")
            identf = singles.tile([128, 128], F32, tag="identf")
            nc.sync.dma_start(out=ident, in_=id_d[:, :])
            nc.sync.dma_start(out=identf, in_=idf_d[:, :])
            g_rep = singles.tile([128, D], F32, tag="g_rep")
            b_rep = singles.tile([128, D], F32, tag="b_rep")
            nc.gpsimd.dma_start(out=g_rep, in_=g_d.ap().partition_broadcast(128))
            nc.gpsimd.dma_start(out=b_rep, in_=b_d.ap().partition_broadcast(128))
            eps_t = singles.tile([128, 1], F32, tag="eps")
            nc.vector.memset(eps_t, EPS)

            x_sup_v = x_d.ap().rearrange("(s j p) f -> s p j f", p=CH, j=SUP)
            y_sup_v = y_d.ap().rearrange("(s j p) f -> s p j f", p=CH, j=SUP)

            for s in range(n_super):
                x_sup = xio.tile([CH, SUP, Q * D], F32, tag="x_sup")
                nc.sync.dma_start(out=x_sup, in_=x_sup_v[s])
                y_sup = yio.tile([CH, SUP, Q * D], F32, tag="y_sup")

                for j in range(SUP):
                    x32 = x_sup[:, j, :]  # [128, 225] fp32

                    # ---- transpose x to fullT ------------------------------
                    psx = ps.tile([128, 256], F32, tag="psx")
                    pxa = psx[:, 0:128]
                    pxb = psx[0:97, 128:256]
                    nc.tensor.transpose(pxa, x32[:, 0:128], identf)
                    nc.tensor.transpose(pxb, x32[:, 128:225], identf)
                    xta = kv.tile([128, 128], F16, tag="xta")
                    xtb = kv.tile([98, 128], F16, tag="xtb")
                    nc.vector.tensor_copy(out=xta, in_=pxa)
                    nc.gpsimd.memset(xtb, 1.0)
                    nc.vector.tensor_copy(out=xtb[0:97, :], in_=pxb)

                    # ---- projections (K fullT, V fullT-dT) -----------------
                    pskv = ps.tile([128, 512], F32, tag="pskv")
                    pka = pskv[:, 0:128]
                    pkb = pskv[0:122, 128:256]
                    pva = pskv[:, 256:384]
                    pvb = pskv[0:106, 384:512]
                    nc.tensor.matmul(pka, wka[:, 0:128], xta, start=True, stop=False)
                    nc.tensor.matmul(pka, wkb[:, 0:128], xtb, start=False, stop=True)
                    nc.tensor.matmul(pkb, wka[:, 128:KF], xta, start=True, stop=False)
                    nc.tensor.matmul(pkb, wkb[:, 128:KF], xtb, start=False, stop=True)
                    nc.tensor.matmul(pva, wva[:, 0:128], xta, start=True, stop=False)
                    nc.tensor.matmul(pva, wvb[:, 0:128], xtb, start=False, stop=True)
                    nc.tensor.matmul(pvb, wva[:, 128:VF], xta, start=True, stop=False)
                    nc.tensor.matmul(pvb, wvb[:, 128:VF], xtb, start=False, stop=True)

                    kta = kv.tile([128, 128], F16, tag="kta")
                    ktb = kv.tile([122, 128], F16, tag="ktb")
                    vta = kv.tile([128, 128], F16, tag="vta")
                    vtb = kv.tile([106, 128], F16, tag="vtb")
                    nc.vector.tensor_copy(out=kta, in_=pka)
                    nc.vector.tensor_copy(out=ktb, in_=pkb)
                    nc.vector.tensor_copy(out=vta, in_=pva)
                    nc.vector.tensor_copy(out=vtb, in_=pvb)

                    # ---- transpose K, V back to batch layout ---------------
                    pst = ps2.tile([128, 512], F16, tag="pst")
                    pkba = pst[:, 0:128]
                    pkbb = pst[:, 128:250]
                    pvba = pst[:, 256:384]
                    pvbb = pst[:, 384:490]
                    nc.tensor.transpose(pkba, kta, ident)
                    nc.tensor.transpose(pkbb, ktb, ident[0:122, 0:122])
                    nc.tensor.transpose(pvba, vta, ident)
                    nc.tensor.transpose(pvbb, vtb, ident[0:106, 0:106])

                    kb = kv.tile([128, KF], F16, tag="kb")
                    vb = kv.tile([128, VF], F16, tag="vb")
                    nc.vector.tensor_copy(out=kb[:, 0:128], in_=pkba)
                    nc.vector.tensor_copy(out=kb[:, 128:KF], in_=pkbb)
                    nc.vector.tensor_copy(out=vb[:, 0:128], in_=pvba)
                    nc.vector.tensor_copy(out=vb[:, 128:VF], in_=pvbb)

                    # ---- scores = K K^T ------------------------------------
                    k3 = kb.rearrange("c (q d) -> c q d", d=DP)       # [128,25,10]
                    prod = big.tile([128, Q, Q, DP], F16, tag="prod")
                    nc.vector.tensor_tensor(
                        out=prod,
                        in0=k3.unsqueeze(2).broadcast_to((128, Q, Q, DP)),
                        in1=k3.unsqueeze(1).broadcast_to((128, Q, Q, DP)),
                        op=OP.mult,
                    )
                    scores = big.tile([128, Q, QP], F32, tag="scores")
                    nc.vector.tensor_reduce(
                        out=scores[:, :, 0:Q],
                        in_=prod[:, :, :, 0:D],
                        axis=AX.X,
                        op=OP.add,
                    )

                    # ---- E = exp(scores - rowmax), row sums, 1/r -----------
                    smax = small.tile([128, Q], F32, tag="smax")
                    nc.vector.tensor_reduce(
                        out=smax, in_=scores[:, :, 0:Q], axis=AX.X, op=OP.max
                    )
                    scs = big.tile([128, Q, QP], F32, tag="scs")
                    nc.gpsimd.tensor_tensor(
                        out=scs[:, :, 0:Q],
                        in0=scores[:, :, 0:Q],
                        in1=smax.unsqueeze(2).broadcast_to((128, Q, Q)),
                        op=OP.subtract,
                    )
                    e_t = big.tile([128, Q, QP], F16, tag="e_t")
                    nc.gpsimd.memset(e_t[:, :, Q:QP], 0.0)
                    nc.scalar.activation(
                        out=e_t[:, :, 0:Q],
                        in_=scs[:, :, 0:Q],
                        func=ACTF.Exp,
                    )
                    rsum = small.tile([128, Q], F32, tag="rsum")
                    nc.vector.tensor_reduce(
                        out=rsum, in_=e_t[:, :, 0:Q], axis=AX.X, op=OP.add
                    )
                    rinv = small.tile([128, Q], F32, tag="rinv")
                    nc.vector.reciprocal(out=rinv, in_=rsum)

                    # ---- res = (E @ V) * rinv ------------------------------
                    v3 = vb.rearrange("c (d qp) -> c d qp", qp=QP)    # [128,9,26]
                    prod2 = big.tile([128, Q, D, QP], F16, tag="prod2")
                    nc.vector.tensor_tensor(
                        out=prod2,
                        in0=e_t.unsqueeze(2).broadcast_to((128, Q, D, QP)),
                        in1=v3.unsqueeze(1).broadcast_to((128, Q, D, QP)),
                        op=OP.mult,
                    )
                    res = small.tile([128, Q, D], F32, tag="res")
                    nc.vector.tensor_reduce(
                        out=res, in_=prod2[:, :, :, 0:Q], axis=AX.X, op=OP.add
                    )

                    # ---- y = x + res/r; LayerNorm over d -------------------
                    y3 = y_sup[:, j, :].rearrange("c (q d) -> c q d", d=D)
                    x3 = x32.rearrange("c (q d) -> c q d", d=D)
                    resn = small.tile([128, Q, D], F32, tag="resn")
                    nc.vector.tensor_tensor(
                        out=resn,
                        in0=res,
                        in1=rinv.unsqueeze(2).broadcast_to((128, Q, D)),
                        op=OP.mult,
                    )
                    yt = small.tile([128, Q, D], F32, tag="yt")
                    nc.vector.tensor_tensor(out=yt, in0=resn, in1=x3, op=OP.add)

                    msum = small.tile([128, Q], F32, tag="msum")
                    nc.vector.tensor_reduce(out=msum, in_=yt, axis=AX.X, op=OP.add)
                    yc = small.tile([128, Q, D], F32, tag="yc")
                    nc.vector.scalar_tensor_tensor(
                        out=yc,
                        in0=msum.unsqueeze(2).broadcast_to((128, Q, D)),
                        scalar=-1.0 / D,
                        in1=yt,
                        op0=OP.mult,
                        op1=OP.add,
                    )
                    sq = small.tile([128, Q, D], F32, tag="sq")
                    nc.vector.tensor_tensor(out=sq, in0=yc, in1=yc, op=OP.mult)
                    vsum = small.tile([128, Q], F32, tag="vsum")
                    nc.vector.tensor_reduce(out=vsum, in_=sq, axis=AX.X, op=OP.add)
                    sd = small.tile([128, Q], F32, tag="sd")
                    nc.scalar.activation(
                        out=sd, in_=vsum, func=ACTF.Sqrt, bias=eps_t, scale=1.0 / D
                    )
                    sdinv = small.tile([128, Q], F32, tag="sdinv")
                    nc.vector.reciprocal(out=sdinv, in_=sd)
                    t2 = small.tile([128, Q, D], F32, tag="t2")
                    nc.gpsimd.tensor_tensor(
                        out=t2,
                        in0=yc,
                        in1=sdinv.unsqueeze(2).broadcast_to((128, Q, D)),
                        op=OP.mult,
                    )
                    t3 = small.tile([128, Q, D], F32, tag="t3")
                    nc.gpsimd.tensor_tensor(
                        out=t3,
                        in0=t2,
                        in1=g_rep.unsqueeze(1).broadcast_to((128, Q, D)),
                        op=OP.mult,
                    )
                    nc.gpsimd.tensor_tensor(
                        out=y3,
                        in0=t3,
                        in1=b_rep.unsqueeze(1).broadcast_to((128, Q, D)),
                        op=OP.add,
                    )

                nc.sync.dma_start(out=y_sup_v[s], in_=y_sup)

    _split_multi_waits(nc)
    return nc


def _host_inputs(inputs, n_super_total=None):
    x = np.ascontiguousarray(np.asarray(inputs["x"], np.float32).reshape(-1, Q * D))
    WK, WV = make_weights(inputs)
    import ml_dtypes

    wk16 = WK.astype(np.float16)
    wv16 = WV.astype(np.float16)
    ident = np.eye(128, dtype=np.float16)
    identf = np.eye(128, dtype=np.float32)
    g = np.asarray(inputs["ln_g"], np.float32)
    b = np.asarray(inputs["ln_b"], np.float32)
    return x, wk16, wv16, ident, identf, g, b


def kernel(**inputs):
    x, wk16, wv16, ident, identf, g, b = _host_inputs(inputs)
    n_el_total = x.shape[0]
    assert n_el_total % (N_CORES * SUP * CH) == 0
    bc = n_el_total // N_CORES
    n_super = bc // (SUP * CH)

    nc = build_nc(n_super)
    in_maps = []
    for i in range(N_CORES):
        in_maps.append(
            {
                "x": x[i * bc : (i + 1) * bc],
                "wk": wk16,
                "wv": wv16,
                "ident": ident,
                "identf": identf,
                "ln_g": g,
                "ln_b": b,
            }
        )
    global LAST_BUILD
    LAST_BUILD = (nc, in_maps)
    rr = run_bass_kernel_spmd(nc, in_maps, list(range(N_CORES)))
    y = np.concatenate([rr.results[i]["y"] for i in range(N_CORES)], axis=0)
    return y.reshape(np.asarray(inputs["x"]).shape)


LAST_BUILD = None



# revision 18
# speedup vs baseline: 1.2676x; 1.2676x over previous
"""Trainium2 Bass kernel for nn_AttentionSubModule (B=262144, Q=25, D=9).

Strategy (pure data parallel over 8 NeuronCores, 32768 elements/core):
  - batch-on-partitions layout, chunks of 128 elements
  - PE: transpose x -> fullT, static block-diag projection matmuls (K, V with
    bias folded via a ones-row), transpose K/V back to batch layout
  - DVE: batched scores = K K^T (broadcast multiply + reduce), attn @ V,
    softmax normalization, residual + LayerNorm
  - ACT: exp (row-max subtracted), sqrt
Dtypes: fp32 I/O and accumulations; fp16 internal operands.
"""

import numpy as np

import bass_rust as br
import concourse.bass as bass
import concourse.mybir as mybir
import concourse.tile as tile
from concourse.bass_utils import run_bass_kernel_spmd
from concourse.vector_clock import ScopedClock

B, Q, D = 262144, 25, 9
SEGS = [(0, 3), (3, 13), (13, 23), (23, 25)]
EPS = 1e-5
N_CORES = 8
BC = B // N_CORES          # elements per core
CH = 128                   # elements per compute chunk
SUP = 8                    # chunks per DMA super-chunk
DP = 10                    # padded d size (even, for 16-bit 2x mode)
QP = 26                    # padded q' size
KF = Q * DP                # 250   K fullT rows / K_b free size
VF = D * QP                # 234   V fullT rows / V_b free size

F32 = mybir.dt.float32
F16 = mybir.dt.float16
AX = mybir.AxisListType
OP = mybir.AluOpType
ACTF = mybir.ActivationFunctionType


def _split_multi_waits(nc, max_waits=1):
    """walrus here rejects instructions with more than one sync-wait command.
    Hoist extra waits onto same-engine NOPs inserted just before the
    offending instruction (same-engine program order makes this equivalent)."""
    for bb in nc.main_func.blocks:
        insts = bb.instructions
        out = []
        changed = False
        for inst in insts:
            si = getattr(inst, "sync_info", None)
            if si is not None and len(si.on_wait) > max_waits:
                waits = list(si.on_wait)
                keep = waits[: max_waits]
                extra = waits[max_waits:]
                for w in extra:
                    nop = mybir.InstNoOp(
                        name=f"wsplit_{nc.next_id()}", ins=[], outs=[]
                    )
                    nop.engine = inst.engine
                    nop.sync_info = br.SyncInfo(on_wait=[w], on_update=[])
                    out.append(nop)
                inst.sync_info = br.SyncInfo(
                    on_wait=keep, on_update=list(si.on_update)
                )
                changed = True
            out.append(inst)
        if changed:
            bb.instructions = out


def _patch_tile_drain():
    """walrus here rejects >1 sync-wait on the Tile tail Drain; spread the
    waits over single-wait NOPs instead."""

    def _drain_and_barrier(self, tick_clock, wait_clock):
        nc = self.nc
        probe = nc.sync.nop(nofuse=True)
        wait_clock.add_sem_waits(
            probe.ins, ScopedClock({None: tick_clock.global_clock})
        )
        si = probe.ins.sync_info
        if si is not None and len(si.on_wait) > 1:
            waits = list(si.on_wait)
            probe.ins.sync_info = br.SyncInfo(
                on_wait=[waits[0]], on_update=list(si.on_update)
            )
            for w in waits[1:]:
                n = nc.sync.nop(nofuse=True)
                n.ins.sync_info = br.SyncInfo(on_wait=[w], on_update=[])
        nc.sync.drain()

        nc.all_engine_barrier()
        assert self.sems is not None
        popped = nc._tile_sem_poison_stack.pop()
        assert popped is self._sem_poison
        nc.clear_and_free_semaphores(list(self.sems.allocated().values()))
        nc.all_engine_barrier()

    tile.TileContext._drain_and_barrier = _drain_and_barrier


_patch_tile_drain()


def _seg_of(q):
    for si, (s, e) in enumerate(SEGS):
        if s <= q < e:
            return si
    raise ValueError(q)


def make_weights(inp):
    """Host-side packing of the static stationary matrices.

    WK [226, 250]: K-proj.  out column m=(q*10+d) [d<9], contraction row
      k=(qt*9+dp) for qt<25 plus bias row k=225.
      WK[qt*9+dp, q*10+d] = Wk_seg(q)[d, dp] * (qt==q);  WK[225, q*10+d] = bk[d]
    WV [226, 234]: V-proj in (d, q')-major output order, m=(d*26+q') [q'<25].
      WV[qt*9+dp, d*26+qp] = Wv_seg(qp)[d, dp] * (qt==qp); WV[225, ...] = bv[d]
    """
    Wk = [np.asarray(inp[n], np.float32) for n in ("W_jk", "W_ok", "W_gk", "W_bk")]
    bk = [np.asarray(inp[n], np.float32) for n in ("b_jk", "b_ok", "b_gk", "b_bk")]
    Wv = [np.asarray(inp[n], np.float32) for n in ("W_jv", "W_ov", "W_gv", "W_bv")]
    bv = [np.asarray(inp[n], np.float32) for n in ("b_jv", "b_ov", "b_gv", "b_bv")]

    WK = np.zeros((226, KF), np.float32)
    WV = np.zeros((226, VF), np.float32)
    for q in range(Q):
        s = _seg_of(q)
        for d in range(D):
            for dp in range(D):
                WK[q * D + dp, q * DP + d] = Wk[s][d, dp]
                WV[q * D + dp, d * QP + q] = Wv[s][d, dp]
            WK[225, q * DP + d] = bk[s][d]
            WV[225, d * QP + q] = bv[s][d]
    return WK, WV


def build_nc(n_super):
    """Build the single-core program processing n_super*SUP*CH elements."""
    n_el = n_super * SUP * CH
    nc = bass.Bass("TRN2", target_bir_lowering=False, debug=False)

    x_d = nc.dram_tensor("x", [n_el, Q * D], F32, kind="ExternalInput")
    y_d = nc.dram_tensor("y", [n_el, Q * D], F32, kind="ExternalOutput")
    wk_d = nc.dram_tensor("wk", [226, KF], F16, kind="ExternalInput")
    wv_d = nc.dram_tensor("wv", [226, VF], F16, kind="ExternalInput")
    id_d = nc.dram_tensor("ident", [128, 128], F16, kind="ExternalInput")
    idf_d = nc.dram_tensor("identf", [128, 128], F32, kind="ExternalInput")
    g_d = nc.dram_tensor("ln_g", [D], F32, kind="ExternalInput")
    b_d = nc.dram_tensor("ln_b", [D], F32, kind="ExternalInput")

    with tile.TileContext(nc) as tc:
        with (
            tc.tile_pool(name="singles", bufs=1) as singles,
            tc.tile_pool(name="xio", bufs=2) as xio,
            tc.tile_pool(name="yio", bufs=2) as yio,
            tc.tile_pool(name="kv", bufs=6) as kv,
            tc.tile_pool(name="big", bufs=3) as big,
            tc.tile_pool(name="small", bufs=8) as small,
            tc.tile_pool(name="ps", bufs=3, space="PSUM") as ps,
            tc.tile_pool(name="ps2", bufs=2, space="PSUM") as ps2,
        ):
            # --- static tiles -------------------------------------------------
            wka = singles.tile([128, KF], F16, tag="wka")
            wkb = singles.tile([98, KF], F16, tag="wkb")
            wva = singles.tile([128, VF], F16, tag="wva")
            wvb = singles.tile([98, VF], F16, tag="wvb")
            nc.sync.dma_start(out=wka, in_=wk_d[0:128, :])
            nc.sync.dma_start(out=wkb[0:98, :], in_=wk_d[128:226, :])
            nc.sync.dma_start(out=wva, in_=wv_d[0:128, :])
            nc.sync.dma_start(out=wvb[0:98, :], in_=wv_d[128:226, :])
            ident = singles.tile([128, 128], F16, tag="ident")
            identf = singles.tile([128, 128], F32, tag="identf")
            nc.sync.dma_start(out=ident, in_=id_d[:, :])
            nc.sync.dma_start(out=identf, in_=idf_d[:, :])
            g_rep = singles.tile([128, D], F32, tag="g_rep")
            b_rep = singles.tile([128, D], F32, tag="b_rep")
            nc.gpsimd.dma_start(out=g_rep, in_=g_d.ap().partition_broadcast(128))
            nc.gpsimd.dma_start(out=b_rep, in_=b_d.ap().partition_broadcast(128))
            eps_t = singles.tile([128, 1], F32, tag="eps")
            nc.vector.memset(eps_t, EPS)

            x_sup_v = x_d.ap().rearrange("(s j p) f -> s p j f", p=CH, j=SUP)
            y_sup_v = y_d.ap().rearrange("(s j p) f -> s p j f", p=CH, j=SUP)

            for s in range(n_super):
                x_sup = xio.tile([CH, SUP, Q * D], F32, tag="x_sup")
                nc.sync.dma_start(out=x_sup, in_=x_sup_v[s])
                y_sup = yio.tile([CH, SUP, Q * D], F32, tag="y_sup")

                for j in range(SUP):
                    x32 = x_sup[:, j, :]  # [128, 225] fp32

                    # ---- transpose x to fullT ------------------------------
                    psx = ps.tile([128, 256], F32, tag="psx")
                    pxa = psx[:, 0:128]
                    pxb = psx[0:97, 128:256]
                    nc.tensor.transpose(pxa, x32[:, 0:128], identf)
                    nc.tensor.transpose(pxb, x32[:, 128:225], identf)
                    xta = kv.tile([128, 128], F16, tag="xta")
                    xtb = kv.tile([98, 128], F16, tag="xtb")
                    nc.vector.tensor_copy(out=xta, in_=pxa)
                    nc.gpsimd.memset(xtb, 1.0)
                    nc.vector.tensor_copy(out=xtb[0:97, :], in_=pxb)

                    # ---- projections (K fullT, V fullT-dT) -----------------
                    pskv = ps.tile([128, 512], F32, tag="pskv")
                    pka = pskv[:, 0:128]
                    pkb = pskv[0:122, 128:256]
                    pva = pskv[:, 256:384]
                    pvb = pskv[0:106, 384:512]
                    nc.tensor.matmul(pka, wka[:, 0:128], xta, start=True, stop=False)
                    nc.tensor.matmul(pka, wkb[:, 0:128], xtb, start=False, stop=True)
                    nc.tensor.matmul(pkb, wka[:, 128:KF], xta, start=True, stop=False)
                    nc.tensor.matmul(pkb, wkb[:, 128:KF], xtb, start=False, stop=True)
                    nc.tensor.matmul(pva, wva[:, 0:128], xta, start=True, stop=False)
                    nc.tensor.matmul(pva, wvb[:, 0:128], xtb, start=False, stop=True)
                    nc.tensor.matmul(pvb, wva[:, 128:VF], xta, start=True, stop=False)
                    nc.tensor.matmul(pvb, wvb[:, 128:VF], xtb, start=False, stop=True)

                    kta = kv.tile([128, 128], F16, tag="kta")
                    ktb = kv.tile([122, 128], F16, tag="ktb")
                    vta = kv.tile([128, 128], F16, tag="vta")
                    vtb = kv.tile([106, 128], F16, tag="vtb")
                    nc.vector.tensor_copy(out=kta, in_=pka)
                    nc.vector.tensor_copy(out=ktb, in_=pkb)
                    nc.vector.tensor_copy(out=vta, in_=pva)
                    nc.vector.tensor_copy(out=vtb, in_=pvb)

                    # ---- transpose K, V back to batch layout ---------------
                    pst = ps2.tile([128, 512], F16, tag="pst")
                    pkba = pst[:, 0:128]
                    pkbb = pst[:, 128:250]
                    pvba = pst[:, 256:384]
                    pvbb = pst[:, 384:490]
                    nc.tensor.transpose(pkba, kta, ident)
                    nc.tensor.transpose(pkbb, ktb, ident[0:122, 0:122])
                    nc.tensor.transpose(pvba, vta, ident)
                    nc.tensor.transpose(pvbb, vtb, ident[0:106, 0:106])

                    kb = kv.tile([128, KF], F16, tag="kb")
                    vb = kv.tile([128, VF], F16, tag="vb")
                    nc.vector.tensor_copy(out=kb[:, 0:128], in_=pkba)
                    nc.vector.tensor_copy(out=kb[:, 128:KF], in_=pkbb)
                    nc.vector.tensor_copy(out=vb[:, 0:128], in_=pvba)
                    nc.vector.tensor_copy(out=vb[:, 128:VF], in_=pvbb)

                    # ---- scores = K K^T ------------------------------------
                    k3 = kb.rearrange("c (q d) -> c q d", d=DP)       # [128,25,10]
                    prod = big.tile([128, Q, Q, DP], F16, tag="prod")
                    nc.vector.tensor_tensor(
                        out=prod,
                        in0=k3.unsqueeze(2).broadcast_to((128, Q, Q, DP)),
                        in1=k3.unsqueeze(1).broadcast_to((128, Q, Q, DP)),
                        op=OP.mult,
                    )
                    scores = big.tile([128, Q, QP], F32, tag="scores")
                    nc.vector.tensor_reduce(
                        out=scores[:, :, 0:Q],
                        in_=prod[:, :, :, 0:D],
                        axis=AX.X,
                        op=OP.add,
                    )

                    # ---- E = exp(scores - rowmax), row sums, 1/r -----------
                    smax = small.tile([128, Q], F32, tag="smax")
                    nc.vector.tensor_reduce(
                        out=smax, in_=scores[:, :, 0:Q], axis=AX.X, op=OP.max
                    )
                    scs = big.tile([128, Q, QP], F32, tag="scs")
                    nc.vector.tensor_tensor(
                        out=scs[:, :, 0:Q],
                        in0=scores[:, :, 0:Q],
                        in1=smax.unsqueeze(2).broadcast_to((128, Q, Q)),
                        op=OP.subtract,
                    )
                    e_t = big.tile([128, Q, QP], F16, tag="e_t")
                    nc.gpsimd.memset(e_t[:, :, Q:QP], 0.0)
                    nc.scalar.activation(
                        out=e_t[:, :, 0:Q],
                        in_=scs[:, :, 0:Q],
                        func=ACTF.Exp,
                    )
                    rsum = small.tile([128, Q], F32, tag="rsum")
                    nc.vector.tensor_reduce(
                        out=rsum, in_=e_t[:, :, 0:Q], axis=AX.X, op=OP.add
                    )
                    rinv = small.tile([128, Q], F32, tag="rinv")
                    nc.vector.reciprocal(out=rinv, in_=rsum)

                    # ---- res = (E @ V) * rinv ------------------------------
                    v3 = vb.rearrange("c (d qp) -> c d qp", qp=QP)    # [128,9,26]
                    prod2 = big.tile([128, Q, D, QP], F16, tag="prod2")
                    nc.vector.tensor_tensor(
                        out=prod2,
                        in0=e_t.unsqueeze(2).broadcast_to((128, Q, D, QP)),
                        in1=v3.unsqueeze(1).broadcast_to((128, Q, D, QP)),
                        op=OP.mult,
                    )
                    res = small.tile([128, Q, D], F32, tag="res")
                    nc.vector.tensor_reduce(
                        out=res, in_=prod2[:, :, :, 0:Q], axis=AX.X, op=OP.add
                    )

                    # ---- y = x + res/r; LayerNorm over d -------------------
                    y3 = y_sup[:, j, :].rearrange("c (q d) -> c q d", d=D)
                    x3 = x32.rearrange("c (q d) -> c q d", d=D)
                    resn = small.tile([128, Q, D], F32, tag="resn")
                    nc.vector.tensor_tensor(
                        out=resn,
                        in0=res,
                        in1=rinv.unsqueeze(2).broadcast_to((128, Q, D)),
                        op=OP.mult,
                    )
                    yt = small.tile([128, Q, D], F32, tag="yt")
                    nc.vector.tensor_tensor(out=yt, in0=resn, in1=x3, op=OP.add)

                    msum = small.tile([128, Q], F32, tag="msum")
                    nc.vector.tensor_reduce(out=msum, in_=yt, axis=AX.X, op=OP.add)
                    yc = small.tile([128, Q, D], F32, tag="yc")
                    nc.vector.scalar_tensor_tensor(
                        out=yc,
                        in0=msum.unsqueeze(2).broadcast_to((128, Q, D)),
                        scalar=-1.0 / D,
                        in1=yt,
                        op0=OP.mult,
                        op1=OP.add,
                    )
                    sq = small.tile([128, Q, D], F32, tag="sq")
                    nc.vector.tensor_tensor(out=sq, in0=yc, in1=yc, op=OP.mult)
                    vsum = small.tile([128, Q], F32, tag="vsum")
                    nc.vector.tensor_reduce(out=vsum, in_=sq, axis=AX.X, op=OP.add)
                    sd = small.tile([128, Q], F32, tag="sd")
                    nc.scalar.activation(
                        out=sd, in_=vsum, func=ACTF.Sqrt, bias=eps_t, scale=1.0 / D
                    )
                    sdinv = small.tile([128, Q], F32, tag="sdinv")
                    nc.vector.reciprocal(out=sdinv, in_=sd)
                    t2 = small.tile([128, Q, D], F32, tag="t2")
                    nc.vector.tensor_tensor(
                        out=t2,
                        in0=yc,
                        in1=sdinv.unsqueeze(2).broadcast_to((128, Q, D)),
                        op=OP.mult,
                    )
                    t3 = small.tile([128, Q, D], F32, tag="t3")
                    nc.gpsimd.tensor_tensor(
                        out=t3,
                        in0=t2,
                        in1=g_rep.unsqueeze(1).broadcast_to((128, Q, D)),
                        op=OP.mult,
                    )
                    nc.gpsimd.tensor_tensor(
                        out=y3,
                        in0=t3,
                        in1=b_rep.unsqueeze(1).broadcast_to((128, Q, D)),
                        op=OP.add,
                    )

                nc.sync.dma_start(out=y_sup_v[s], in_=y_sup)

    _split_multi_waits(nc)
    return nc


def _host_inputs(inputs, n_super_total=None):
    x = np.ascontiguousarray(np.asarray(inputs["x"], np.float32).reshape(-1, Q * D))
    WK, WV = make_weights(inputs)
    import ml_dtypes

    wk16 = WK.astype(np.float16)
    wv16 = WV.astype(np.float16)
    ident = np.eye(128, dtype=np.float16)
    identf = np.eye(128, dtype=np.float32)
    g = np.asarray(inputs["ln_g"], np.float32)
    b = np.asarray(inputs["ln_b"], np.float32)
    return x, wk16, wv16, ident, identf, g, b


def kernel(**inputs):
    x, wk16, wv16, ident, identf, g, b = _host_inputs(inputs)
    n_el_total = x.shape[0]
    assert n_el_total % (N_CORES * SUP * CH) == 0
    bc = n_el_total // N_CORES
    n_super = bc // (SUP * CH)

    nc = build_nc(n_super)
    in_maps = []
    for i in range(N_CORES):
        in_maps.append(
            {
                "x": x[i * bc : (i + 1) * bc],
                "wk": wk16,
                "wv": wv16,
                "ident": ident,
                "identf": identf,
                "ln_g": g,
                "ln_b": b,
            }
        )
    global LAST_BUILD
    LAST_BUILD = (nc, in_maps)
    rr = run_bass_kernel_spmd(nc, in_maps, list(range(N_CORES)))
    y = np.concatenate([rr.results[i]["y"] for i in range(N_CORES)], axis=0)
    return y.reshape(np.asarray(inputs["x"]).shape)


LAST_BUILD = None

